# revision 28
# baseline (speedup 1.0000x reference)
"""BiDAF attention-flow kernel for Trainium2 (8 NeuronCores, data-parallel over batch).

Per core (one batch element):
  s[j,i]   = c[j] + q[i] + sum_h w_cq[h]*emb2[j,h]*emb1[i,h]
  a        = softmax_i(s)          (c[j] drops out of the row softmax)
  y2x      = a @ emb1
  b_att    = softmax_j(max_i s)
  x2y      = sum_j b_att[j]*emb2[j]
  out      = [emb2, y2x, emb2*y2x, emb2*x2y] @ w_red + b_red

Implementation notes:
  - b_c/b_q/b_cq cancel exactly in both softmaxes (row/column constants).
  - Row softmax uses a FIXED exp shift (s - SHIFT); true row max recovered as
    SHIFT + ln(max_i u) for b_att.
  - The s-matmul, y2x and pass-1 run on fp8(e4m3) with DoubleRow perf mode
    (2 K-planes per instruction).  Weights are pre-scaled x16 on the host so
    their fp8 encodings stay in the normal range; the 1/16 descale is folded
    into the exp scale / output STTs.
  - Softmax normalization is folded into the PE transpose of u: transposing
    against diag(128/Z_j) instead of the identity yields normalized a^T free
    (the x128 keeps small attention weights out of fp8 denormals).
  - ALL data layouts are prepared host-side: q_score/c_score rows (0.4% of
    the FLOPs, exact f32), fp8 casts of emb1/emb2 in both transposed and
    natural layouts, and partition-major packing so every DMA lands as 128
    contiguous multi-KB rows (descriptor-count minimized).  This removes all
    on-device q/c matmuls, natural-layout PE transposes and fp8 cast traffic
    that used to serialize the load phase.
  - DMA issue order is a priority schedule: the first e2 j-block and first
    e1 i-slab go first so the s-matmul pipeline starts within ~8us; the
    bulk (e1n, e2tt, e2n, weights) streams in underneath the main loop.
  - x2y is computed in natural orientation (stationary = b_att column, so
    LDWEIGHTS is 1 column instead of 128) as 2x16 N=384 accumulating
    matmuls, then 6 tiny K=1 transposes -- instead of 96 N=1 matmuls.
  - Main loop is software-pipelined (A = s/exp/stats, T = u transposes, Y =
    grouped y2x, C = pass-1) so the in-order PE queue never head-blocks on a
    fresh dependency; idle >3.4us would re-throttle the HAM clock to 1.2 GHz.
"""

import numpy as np
import ml_dtypes

P = 128
XL = 2048
YL = 2048
H = 768
OUT = 300
OUTP = 320      # OUT padded to a 16B-aligned fp8 stride for DoubleRow
NJT = YL // P   # 16 j tiles
NIC = XL // P   # 16 i chunks
NHC = H // P    # 6 h chunks
SLAB = 512
NSLAB = XL // SLAB  # 4
NG = NJT // 4   # 4 j-tile groups
NCORES = 8
SHIFT = 2.0     # fixed exp shift; keeps u = exp(s-SHIFT) in bf16 range
WS = 16.0       # host-side weight scale (wq, wcq, w1..w4)
BS = 64.0       # b_att fp8 scale

_CACHE = {}


def _fix_waits(nc, mybir, max_waits=1):
    """This walrus build rejects >1 sync wait per instruction.

    Pass 1: drop waits that are transitively implied by another wait on the
    same instruction.  Pass 2: hoist remaining extra waits onto same-engine
    NoOps inserted right before the instruction.
    """
    from collections import defaultdict

    blocks = [bb for f in nc.m.functions for bb in f.blocks]
    insts = [ins for bb in blocks for ins in bb.instructions]

    dma_types = ("InstDMACopy", "InstDmaTransposeAnt")
    eng_stream = defaultdict(list)
    queue_stream = defaultdict(list)
    sem_events = defaultdict(list)
    cum = defaultdict(int)
    for i, ins in enumerate(insts):
        eng_stream[str(ins.engine)].append(i)
        si = ins.sync_info
        if si and si.on_update:
            for u in si.on_update:
                cum[u.id] += u.update_value
                sem_events[u.id].append((cum[u.id], i))
                if type(ins).__name__ in dma_types:
                    queue_stream[u.id].append(i)

    def achiever(sem_id, val):
        for cv, i in sem_events.get(sem_id, []):
            if cv >= val:
                return i
        return None

    eng_pos, q_pos = {}, {}
    for e, lst in eng_stream.items():
        for k, i in enumerate(lst):
            eng_pos[i] = (e, k)
    for s, lst in queue_stream.items():
        for k, i in enumerate(lst):
            q_pos[i] = (s, k)

    memo = {}

    def implied(i):
        if i in memo:
            return memo[i]
        memo[i] = set()
        out = {i}
        ins = insts[i]
        if i in q_pos:
            s, k = q_pos[i]
            if k > 0:
                out |= implied(queue_stream[s][k - 1])
        e, k = eng_pos[i]
        j = k - 1
        while j >= 0:
            p = eng_stream[e][j]
            if type(insts[p]).__name__ in dma_types:
                j -= 1
                continue
            out |= implied(p)
            break
        si = ins.sync_info
        if si and si.on_wait:
            for w in si.on_wait:
                a = achiever(w.id, w.wait_value)
                if a is not None:
                    out |= implied(a)
        memo[i] = out
        return out

    # pass 1: redundancy elimination
    for i, ins in enumerate(insts):
        si = ins.sync_info
        if not (si and si.on_wait and len(si.on_wait) > max_waits):
            continue
        waits = list(si.on_wait)
        ach = [(w, achiever(w.id, w.wait_value)) for w in waits]
        keep = []
        for wi, (w, a) in enumerate(ach):
            red = False
            if a is not None:
                for wj, (w2, a2) in enumerate(ach):
                    if wi != wj and a2 is not None and a != a2 and a in implied(a2):
                        red = True
                        break
            if not red:
                keep.append(w)
        si.on_wait = keep

    # pass 2: hoist extras onto same-engine NoOps
    k = 0
    for bb in blocks:
        lst = bb.instructions
        i = 0
        while i < len(lst):
            ins = lst[i]
            si = ins.sync_info
            if si and si.on_wait and len(si.on_wait) > max_waits:
                waits = list(si.on_wait)
                extra, keep = waits[:-max_waits], waits[-max_waits:]
                si.on_wait = keep
                nops = []
                for w in extra:
                    nop = mybir.InstNoOp(name=f"I-waitfix-{k}", ins=[], outs=[])
                    k += 1
                    nop.engine = ins.engine
                    nop.sync_info = mybir.SyncInfo(on_wait=[w], on_update=[])
                    nops.append(nop)
                lst[i:i] = nops
                i += len(nops)
            i += 1


def _build():
    import concourse.bass as bass
    import concourse.tile as tile
    import concourse.mybir as mybir
    from concourse.masks import make_identity

    f32 = mybir.dt.float32
    bf16 = mybir.dt.bfloat16
    fp8 = mybir.dt.float8e4
    DR = mybir.MatmulPerfMode.DoubleRow
    MUL = mybir.AluOpType.mult
    ADD = mybir.AluOpType.add
    MAX = mybir.AluOpType.max
    EXP = mybir.ActivationFunctionType.Exp
    LN = mybir.ActivationFunctionType.Ln
    AXX = mybir.AxisListType.X

    nc = bass.Bass("TRN2", target_bir_lowering=False, debug=False,
                   num_devices=NCORES)

    # All big tensors arrive host-packed partition-major: DRAM row p holds
    # everything SBUF partition p needs, so each dma_start is 128 large
    # contiguous descriptors.
    e2ts_d = nc.dram_tensor("e2ts", [P, NJT * NHC * P], fp8,
                            kind="ExternalInput")       # [p, jt, hc, j]
    e1ts_d = nc.dram_tensor("e1ts", [P, NSLAB * NHC * SLAB], fp8,
                            kind="ExternalInput")       # [p, sl, hc, i]
    e1n_d = nc.dram_tensor("e1n", [P, NIC * H], fp8,
                           kind="ExternalInput")        # [p, ic, h]
    e2n_d = nc.dram_tensor("e2n", [P, NJT * H], fp8,
                           kind="ExternalInput")        # [p, jt, h]
    e2tt_d = nc.dram_tensor("e2tt", [P, NG * NHC * SLAB], bf16,
                            kind="ExternalInput")       # [p, g, hc, j]
    qrow_d = nc.dram_tensor("qrow", [1, XL], bf16, kind="ExternalInput")
    c_d = nc.dram_tensor("crow", [P, NJT], f32, kind="ExternalInput")
    w23q_d = nc.dram_tensor("w23q", [P, NHC * 2 * OUTP], fp8,
                            kind="ExternalInput")       # [p, hc, {w2,w3}, o]
    w14s_d = nc.dram_tensor("w14s", [P, NHC * 2 * OUTP], bf16,
                            kind="ExternalInput")       # [p, hc, {w1,w4}, o]
    bred_d = nc.dram_tensor("bred", [1, OUTP], f32, kind="ExternalInput")
    out_d = nc.dram_tensor("out", [YL, OUT], bf16, kind="ExternalOutput")

    with tile.TileContext(nc) as tc:
        with (
            tc.tile_pool(name="res", bufs=1) as res,        # resident data
            tc.tile_pool(name="stage", bufs=3) as stage,    # out staging
            tc.tile_pool(name="small", bufs=1) as small,    # stats etc
            tc.tile_pool(name="pss", bufs=2, space="PSUM") as pss,
            tc.tile_pool(name="ptp", bufs=2, space="PSUM") as ptp,
            tc.tile_pool(name="psy", bufs=2, space="PSUM") as psy,
            tc.tile_pool(name="pso", bufs=2, space="PSUM") as pso,
        ):
            # ---- constants ----
            ident16 = res.tile([P, P], bf16, tag="ident16")
            make_identity(nc, ident16)
            ident32 = res.tile([P, P], f32, tag="ident32")
            make_identity(nc, ident32)
            onesBS = res.tile([1, P], f32, tag="onesBS")
            nc.vector.memset(onesBS, BS)
            ones16 = res.tile([1, P], bf16, tag="ones16")
            nc.vector.memset(ones16, 1.0)
            ones11 = res.tile([1, 1], bf16, tag="ones11")
            nc.vector.memset(ones11, 1.0)
            identUS = res.tile([P, P], bf16, tag="identUS")
            nc.vector.tensor_scalar_mul(identUS, ident16, 128.0)
            negC = res.tile([P, 1], f32, tag="negC")
            nc.vector.memset(negC, -SHIFT)
            warm_sb = res.tile([P, OUTP], bf16, tag="warm_sb")
            nc.vector.memset(warm_sb, 0.0)

            # PE warm-up with REAL matmuls: HAM grants 2.4 GHz only after
            # ~3.4us of sustained activity; these bridge t=0 until the first
            # s-matmuls take over.
            _warm_k = [0]

            def warm(n):
                for _ in range(n):
                    wk = _warm_k[0]
                    _warm_k[0] += 1
                    wps = pso.tile([P, OUTP], f32, tag="pso", name=f"warm{wk}")
                    nc.tensor.matmul(wps, ident16, warm_sb, start=True,
                                     stop=True, skip_group_check=True)

            # enough to keep the PE continuously busy from ~8us until the
            # first e1ts slab lands (~12-13us, DMA-bound): HAM then grants
            # 2.4 GHz at ~11.5us and the whole prologue runs warm
            warm(18)

            # ---- resident tiles ----
            e2ts = res.tile([P, NJT, NHC, P], fp8, tag="e2ts")
            e1ts = res.tile([P, NSLAB, NHC, SLAB], fp8, tag="e1ts")
            e1n = res.tile([P, NIC, H], fp8, tag="e1n")
            e2n = res.tile([P, NJT, H], fp8, tag="e2n")
            e2tt = res.tile([P, NG, NHC, SLAB], bf16, tag="e2tt")
            qrow = small.tile([1, XL], bf16, tag="qrow")
            c_sb = small.tile([P, NJT], f32, tag="c_sb")
            w23q = res.tile([P, NHC, 2, OUTP], fp8, tag="w23q")
            w14s = res.tile([P, NHC, 2, OUTP], bf16, tag="w14s")
            bred_bc = res.tile([P, OUTP], f32, tag="bred_bc")

            # ---- DMA priority schedule ----
            # SDMA engines drain all queued DMAs in packet round-robin, so
            # issuing the bulk early steals bandwidth from the loop-critical
            # transfers.  Two-stage pacing: the critical set (e2ts-g0 +
            # e1ts slabs, 1.9MB) goes alone first; the bulk is chained
            # behind it with tiny gpsimd pre-writes on the DMA destinations
            # (WAW dependency delays the DMA issue until the pacer fires).
            JTW = NHC * P          # dram cols per j-tile of e2ts
            SLW = NHC * SLAB       # dram cols per i-slab of e1ts
            GW = NHC * SLAB        # dram cols per group of e2tt
            nc.sync.dma_start(out=e2ts[:, 0:4, :, :],
                              in_=e2ts_d[:, 0:4 * JTW])
            for sl in range(NSLAB):
                nc.sync.dma_start(out=e1ts[:, sl, :, :],
                                  in_=e1ts_d[:, sl * SLW:(sl + 1) * SLW])
            # scalar ring: ONLY tiny transfers (a big one would head-block
            # the scalar queue and delay the exps queued behind it)
            nc.scalar.dma_start(out=qrow, in_=qrow_d[:])
            nc.scalar.dma_start(out=c_sb, in_=c_d[:])
            _bap = bred_d.ap()
            nc.scalar.dma_start(out=bred_bc, in_=bass.AP(
                tensor=_bap.tensor, offset=_bap.offset,
                ap=[[0, P]] + list(_bap.ap[1:])))

            paceA = small.tile([1, 1], f32, tag="paceA")
            paceB = small.tile([1, 1], f32, tag="paceB")
            # stage A fires once critical slab 2 has landed (slab 3 rides
            # alongside A's start)
            nc.gpsimd.tensor_copy(out=paceA, in_=e1ts[0:1, 2, 0, 0:1])

            def paced_dma(pace, out_corner, out, in_):
                nc.gpsimd.tensor_copy(out=out_corner, in_=pace)
                nc.sync.dma_start(out=out, in_=in_)

            # stage A in need-order: e1n (y(0) matmuls), e2tt-g0 (y(0)
            # drains), next e2ts groups, pass-1 weights
            for b in range(4):
                paced_dma(paceA, e1n[0:1, 4 * b, 0:1],
                          e1n[:, 4 * b:4 * (b + 1), :],
                          e1n_d[:, 4 * b * H:4 * (b + 1) * H])
            paced_dma(paceA, e2tt[0:1, 0, 0, 0:1], e2tt[:, 0, :, :],
                      e2tt_d[:, 0:GW])
            paced_dma(paceA, w23q[0:1, 0, 0, 0:1], w23q, w23q_d[:])
            for g in range(1, NG):
                paced_dma(paceA, e2ts[0:1, 4 * g, 0, 0:1],
                          e2ts[:, 4 * g:4 * (g + 1), :, :],
                          e2ts_d[:, 4 * g * JTW:4 * (g + 1) * JTW])
            # stage B fires once the last stage-A tensor has landed
            nc.gpsimd.tensor_copy(out=paceB, in_=e2ts[0:1, 4 * (NG - 1), 0, 0:1])
            for g in range(1, NG):
                paced_dma(paceB, e2tt[0:1, g, 0, 0:1], e2tt[:, g, :, :],
                          e2tt_d[:, g * GW:(g + 1) * GW])
            paced_dma(paceB, w14s[0:1, 0, 0, 0:1], w14s, w14s_d[:])
            for b in range(4):
                paced_dma(paceB, e2n[0:1, 4 * b, 0:1],
                          e2n[:, 4 * b:4 * (b + 1), :],
                          e2n_d[:, 4 * b * H:4 * (b + 1) * H])

            # ---- stats tiles ----
            M_sb = small.tile([P, NJT], f32, tag="M")
            Z_sb = small.tile([P, NJT], f32, tag="Z")
            rZ_sb = small.tile([P, NJT], f32, tag="rZ")
            out_sb = res.tile([P, NJT, OUTP], f32, tag="out_sb")

            # ---- main loop: software-pipelined, y2x grouped by 4 tiles ----
            sjt_cm = tc.tile_pool(name="sjt", bufs=2)
            sjt = sjt_cm.__enter__()
            sg4_cm = tc.tile_pool(name="sg4", bufs=2)
            sg4 = sg4_cm.__enter__()
            tiles = {}
            gtiles = {}

            def c_mm_pair(jt, k, op1):
                # k-th pair of tile jt's 6 pass-1 DR matmuls ([y2x;e2*y2x] @
                # 16[w2;w3]).  Hosted between a-phase slabs so their 213ns
                # DoubleRow LDWEIGHTS hide under the 216ns s-matmul streams.
                g4, jj = jt // 4, jt % 4
                y2xT4, bl34 = gtiles[(g4, "y")]
                jsl4 = slice(jj * P, (jj + 1) * P)
                for m in (2 * k, 2 * k + 1):
                    sweep, hp = divmod(m, 3)
                    src = y2xT4 if sweep == 0 else bl34
                    nc.tensor.matmul(
                        op1, src[:, 2 * hp:2 * hp + 2, jsl4],
                        w23q[:, 2 * hp:2 * hp + 2, sweep, :],
                        start=(m == 0), stop=(m == 5),
                        perf_mode=DR, skip_group_check=True)

            def a_phase(jt, fillers=0, fast_m=False, host_c=None):
                # 16*s = 16*q + (e2*16wcq) @ e1^T; u = exp(16s/16 - SHIFT)
                u = sjt.tile([P, XL], bf16, tag="u", name=f"u{jt}")
                Zp = sjt.tile([P, NSLAB], f32, tag="Zp", name=f"Zp{jt}")
                if host_c is not None:
                    cop1 = pso.tile([P, OUTP], f32, tag="pso",
                                    name=f"op1_{host_c}")
                if fast_m:
                    mx4 = sjt.tile([P, NSLAB], f32, tag="mx4", name=f"mx4{jt}")
                for sl in range(NSLAB):
                    ssl = slice(sl * SLAB, (sl + 1) * SLAB)
                    sp = pss.tile([P, SLAB], f32, tag="pss", name=f"sp{jt}_{sl}")
                    # q-broadcast CLOSES the group so the DR matmuls can
                    # start as soon as this e1ts slab's DMA lands
                    for hp in range(NHC // 2):
                        nc.tensor.matmul(
                            sp, e2ts[:, jt, 2 * hp:2 * hp + 2, :],
                            e1ts[:, sl, 2 * hp:2 * hp + 2, :],
                            start=(hp == 0), stop=False,
                            perf_mode=DR, skip_group_check=True)
                    nc.tensor.matmul(sp, ones16, qrow[:, ssl],
                                     start=False, stop=True,
                                     skip_group_check=True)
                    nc.scalar.activation(out=u[:, ssl], in_=sp, func=EXP,
                                         bias=negC, scale=1.0 / WS,
                                         accum_out=Zp[:, sl:sl + 1])
                    if fast_m:
                        # last tile: per-slab row max of 16s from PSUM (max
                        # commutes with exp) so M(15) -- which gates the
                        # whole b_att epilogue chain -- is ready right away
                        nc.vector.tensor_reduce(out=mx4[:, sl:sl + 1],
                                                in_=sp, axis=AXX, op=MAX)
                    if host_c is not None and sl < 3:
                        c_mm_pair(host_c, sl, cop1)
                    # DMA-wait fillers: burn PE cycles on the slab that just
                    # landed so HAM never sees a >3.4us idle window while
                    # the next slab's DMA is in flight (prologue only).
                    # Reading e1ts keeps them dependency-paced -- the
                    # scheduler cannot hoist them to t=0.
                    if fillers and sl < NSLAB - 1:
                        for k in range(fillers):
                            wk = _warm_k[0]
                            _warm_k[0] += 1
                            wps = pso.tile([P, OUTP], f32, tag="pso",
                                           name=f"fill{wk}")
                            nc.tensor.matmul(
                                wps, ident16,
                                e1ts[:, sl, k % NHC, 0:OUTP],
                                start=True, stop=True,
                                skip_group_check=True)
                # Z -> 1/Z -> diag first: the t-phase transposes head-block
                # the in-order PE queue on diag, so keep its chain short
                # (umax/M go AFTER on the in-order vector queue)
                nc.vector.tensor_reduce(out=Z_sb[:, jt:jt + 1], in_=Zp,
                                        axis=AXX, op=ADD)
                nc.vector.reciprocal(out=rZ_sb[:, jt:jt + 1],
                                     in_=Z_sb[:, jt:jt + 1])
                # normalized a^T transpose operand: diag(128/Z_j) -- the
                # x128 keeps small attention weights out of fp8 denormals
                diag = sjt.tile([P, P], bf16, tag="diag", name=f"diag{jt}")
                nc.vector.tensor_scalar_mul(diag, identUS, rZ_sb[:, jt:jt + 1])

                if fast_m:
                    # M = c + max(16s)/16, exact
                    mx = sjt.tile([P, 1], f32, tag="mx", name=f"mx{jt}")
                    nc.vector.tensor_reduce(out=mx, in_=mx4, axis=AXX, op=MAX)
                    nc.vector.scalar_tensor_tensor(
                        out=M_sb[:, jt:jt + 1], in0=mx, scalar=1.0 / WS,
                        in1=c_sb[:, jt:jt + 1], op0=MUL, op1=ADD)
                else:
                    # row max for b_att: M = c + SHIFT + ln(max u)
                    umax = sjt.tile([P, 1], f32, tag="umax", name=f"umax{jt}")
                    nc.vector.tensor_reduce(out=umax, in_=u, axis=AXX, op=MAX)
                    lnu = sjt.tile([P, 1], f32, tag="lnu", name=f"lnu{jt}")
                    nc.scalar.activation(out=lnu, in_=umax, func=LN)
                    nc.vector.scalar_tensor_tensor(
                        out=M_sb[:, jt:jt + 1], in0=lnu, scalar=SHIFT,
                        in1=c_sb[:, jt:jt + 1], op0=ADD, op1=ADD)
                if host_c is not None:
                    nc.vector.scalar_tensor_tensor(
                        out=out_sb[:, host_c, :], in0=cop1, scalar=1.0 / WS,
                        in1=bred_bc, op0=MUL, op1=ADD)
                    if host_c % 4 == 3:
                        gtiles.pop((host_c // 4, "y"))
                tiles[jt] = (u, diag)

            def get_uT4(g):
                if g not in gtiles:
                    gtiles[g] = sg4.tile([P, NIC, 4, P], fp8, tag="uT4",
                                         name=f"uT4_{g}")
                return gtiles[g]

            def t_phase(jt):
                u, diag = tiles.pop(jt)
                uT4 = get_uT4(jt // 4)
                jj = jt % 4
                # uT4[i, ic, jj, j] = u[j, i] * 128*rZ_j, fp8 cast on the copy
                for g in range(NIC // 4):
                    tp = ptp.tile([P, 4, P], f32, tag="ptp", name=f"tp{jt}_{g}")
                    for k in range(4):
                        ic = g * 4 + k
                        nc.tensor.matmul(tp[:, k, :], u[:, ic * P:(ic + 1) * P],
                                         diag, start=True, stop=True,
                                         skip_group_check=True)
                    nc.any.tensor_copy(out=uT4[:, g * 4:(g + 1) * 4, jj, :],
                                       in_=tp)

            def y_phase(g):
                uT4 = gtiles.pop(g)
                y2xT4 = sg4.tile([P, NHC, 4 * P], fp8, tag="y2xT4",
                                 name=f"y2xT4_{g}")
                bl34 = sg4.tile([P, NHC, 4 * P], fp8, tag="bl34",
                                name=f"bl34_{g}")
                for hc in range(NHC):
                    yp = psy.tile([P, 4 * P], f32, tag="psy", name=f"yp{g}_{hc}")
                    for icp in range(NIC // 2):
                        nc.tensor.matmul(
                            yp,
                            e1n[:, 2 * icp:2 * icp + 2, hc * P:(hc + 1) * P],
                            uT4[:, 2 * icp:2 * icp + 2, :, :],
                            start=(icp == 0), stop=(icp == NIC // 2 - 1),
                            perf_mode=DR, skip_group_check=True)
                    # both drains read yp (PSUM) independently, so the
                    # scheduler can run them on different engines
                    nc.any.tensor_scalar_mul(y2xT4[:, hc, :], yp, 1.0 / 128.0)
                    nc.vector.scalar_tensor_tensor(
                        out=bl34[:, hc, :], in0=yp, scalar=1.0 / 128.0,
                        in1=e2tt[:, g, hc, :], op0=MUL, op1=MUL)
                gtiles[(g, "y")] = (y2xT4, bl34)

            def c_phase(jt):
                g, jj = jt // 4, jt % 4
                y2xT4, bl34 = gtiles[(g, "y")]
                jsl4 = slice(jj * P, (jj + 1) * P)
                # pass-1 reduction: [y2x; e2*y2x] @ 16*[w2; w3] (DoubleRow)
                op1 = pso.tile([P, OUTP], f32, tag="pso", name=f"op1_{jt}")
                for hp in range(NHC // 2):
                    nc.tensor.matmul(op1, y2xT4[:, 2 * hp:2 * hp + 2, jsl4],
                                     w23q[:, 2 * hp:2 * hp + 2, 0, :],
                                     start=(hp == 0), stop=False,
                                     perf_mode=DR, skip_group_check=True)
                for hp in range(NHC // 2):
                    nc.tensor.matmul(op1, bl34[:, 2 * hp:2 * hp + 2, jsl4],
                                     w23q[:, 2 * hp:2 * hp + 2, 1, :],
                                     start=False, stop=(hp == NHC // 2 - 1),
                                     perf_mode=DR, skip_group_check=True)
                # out_sb = psum/16 + b_red
                nc.vector.scalar_tensor_tensor(
                    out=out_sb[:, jt, :], in0=op1, scalar=1.0 / WS,
                    in1=bred_bc, op0=MUL, op1=ADD)
                if jj == 3:
                    gtiles.pop((g, "y"))

            # prologue: A0..A4 interleaved with T0..T3; warm filler covers
            # the DMA trickle so the HAM clock never re-throttles
            a_phase(0, fillers=5)
            warm(3)
            for jt in range(1, 5):
                a_phase(jt)
                warm(2)
                t_phase(jt - 1)
            post_cm = tc.tile_pool(name="post", bufs=1)
            post = post_cm.__enter__()
            for g in range(4):
                y_phase(g)
                if g < 3:
                    for jj in range(4):
                        jt = 4 * (g + 1) + jj
                        if jt + 1 < NJT:
                            a_phase(jt + 1, host_c=4 * g + jj)
                        else:
                            c_phase(4 * g + jj)  # c(11): no a-phase left
                        t_phase(jt)
                else:
                    # epilogue: b_att chain + x2y overlap the last c-phases
                    # ---- b_att = softmax_j(M), no max shift ----
                    bexp = post.tile([P, NJT], f32, tag="bexp")
                    brow = post.tile([P, 1], f32, tag="brow")
                    nc.scalar.activation(out=bexp, in_=M_sb, func=EXP,
                                         accum_out=brow)
                    tpb = pss.tile([1, P], f32, tag="pss", name="tpb")
                    nc.tensor.transpose(tpb, brow, ident32)
                    brw = post.tile([1, P], f32, tag="brw")
                    nc.vector.tensor_copy(out=brw, in_=tpb)
                    bs0 = post.tile([1, 1], f32, tag="bs0")
                    nc.vector.tensor_reduce(out=bs0, in_=brw, axis=AXX, op=ADD)
                    rb0 = post.tile([1, 1], f32, tag="rb0")
                    nc.vector.reciprocal(rb0, bs0)
                    rbp = pss.tile([P, 1], f32, tag="pss", name="rbp")
                    nc.tensor.matmul(rbp, onesBS, rb0, start=True, stop=True,
                                     skip_group_check=True)
                    rbz = post.tile([P, 1], f32, tag="rbz")
                    nc.vector.tensor_copy(out=rbz, in_=rbp)
                    battq = post.tile([P, NJT], fp8, tag="battq")
                    nc.vector.tensor_scalar_mul(battq, bexp, rbz)

                    c_phase(12)
                    c_phase(13)

                    # x2y natural orientation: stationary = b_att column
                    # (LDWEIGHTS is 1 column), moving = e2n rows.
                    # x2row[0, h] = sum_j (64 b_j) e2[j, h], split in 2 PSUM
                    # halves of N=384.
                    HH = H // 2
                    x2pa = psy.tile([1, HH], f32, tag="psy", name="x2pa")
                    x2pb = psy.tile([1, HH], f32, tag="psy", name="x2pb")
                    for jc in range(NJT):
                        nc.tensor.matmul(
                            x2pa, battq[:, jc:jc + 1], e2n[:, jc, 0:HH],
                            start=(jc == 0), stop=(jc == NJT - 1),
                            skip_group_check=True)
                        nc.tensor.matmul(
                            x2pb, battq[:, jc:jc + 1], e2n[:, jc, HH:H],
                            start=(jc == 0), stop=(jc == NJT - 1),
                            skip_group_check=True)
                    c_phase(14)
                    # x2row bf16 (descale by 1/BS), then 6 tiny K=1
                    # transposes to get x2y as per-partition scalars
                    x2row = post.tile([1, H], bf16, tag="x2row")
                    nc.vector.tensor_scalar_mul(x2row[:, 0:HH], x2pa, 1.0 / BS)
                    nc.vector.tensor_scalar_mul(x2row[:, HH:H], x2pb, 1.0 / BS)
                    x2p6 = pss.tile([P, NHC], f32, tag="pss", name="x2p6")
                    for hc in range(NHC):
                        nc.tensor.matmul(
                            x2p6[:, hc:hc + 1],
                            x2row[:, hc * P:(hc + 1) * P], ones11,
                            start=True, stop=True, skip_group_check=True)
                    c_phase(15)
                    x2yT = post.tile([P, NHC], f32, tag="x2yT")
                    nc.vector.tensor_copy(out=x2yT, in_=x2p6)

                    # w14 = 16*(w1 + x2y*w4), bf16
                    w14 = res.tile([P, NHC, OUTP], bf16, tag="w14")
                    for hc in range(NHC):
                        nc.vector.scalar_tensor_tensor(
                            out=w14[:, hc, :],
                            in0=w14s[:, hc, 1, :],
                            scalar=x2yT[:, hc:hc + 1],
                            in1=w14s[:, hc, 0, :],
                            op0=MUL, op1=ADD)
                    warm(4)

            post_cm.__exit__(None, None, None)
            sg4_cm.__exit__(None, None, None)
            sjt_cm.__exit__(None, None, None)

            # ---- pass 2: out += emb2 @ w14/16 (bf16), stream out ----
            for jt in range(NJT):
                g, jj = jt // 4, jt % 4
                jsl = slice(jt * P, (jt + 1) * P)
                op2 = pso.tile([P, OUTP], f32, tag="pso", name=f"op2_{jt}")
                for hc in range(NHC):
                    nc.tensor.matmul(
                        op2, e2tt[:, g, hc, jj * P:(jj + 1) * P],
                        w14[:, hc, :],
                        start=(hc == 0), stop=(hc == NHC - 1),
                        skip_group_check=True)
                fin = stage.tile([P, OUTP], bf16, tag="fin", name=f"fin{jt}")
                nc.vector.scalar_tensor_tensor(
                    out=fin, in0=op2, scalar=1.0 / WS,
                    in1=out_sb[:, jt, :], op0=MUL, op1=ADD)
                nc.sync.dma_start(out=out_d[jsl, :], in_=fin[:, 0:OUT])

    return nc


def _get_nc(drain_fix=True):
    if "nc" not in _CACHE:
        _CACHE["nc"] = _build()
    if drain_fix and not _CACHE.get("drain_fixed"):
        import concourse.mybir as mybir
        _fix_waits(_CACHE["nc"], mybir, max_waits=1)
        _CACHE["drain_fixed"] = True
    return _CACHE["nc"]


def _prep_inputs(emb1, emb2, w_c, w_q, w_cq, w_red, b_red):
    """Host-side prep: exact q/c score rows, fp8/bf16 casts and
    partition-major packing of all embedding layouts."""
    bf = ml_dtypes.bfloat16
    f8 = ml_dtypes.float8_e4m3

    emb1 = np.asarray(emb1, np.float32)     # [B, XL, H]
    emb2 = np.asarray(emb2, np.float32)     # [B, YL, H]
    w_c = np.asarray(w_c, np.float32)
    w_q = np.asarray(w_q, np.float32)
    w_cq = np.asarray(w_cq, np.float32)
    w_red = np.asarray(w_red, np.float32)
    b_red = np.asarray(b_red, np.float32)
    B = emb1.shape[0]

    # exact score rows (0.4% of the kernel FLOPs, done in f32 on host)
    qrow = ((emb1 @ w_q) * WS).astype(bf)           # [B, XL]
    crow = emb2 @ w_c                               # [B, YL]
    c_sb = np.ascontiguousarray(
        crow.reshape(B, NJT, P).transpose(0, 2, 1))  # [B, P, NJT]

    # transposed fp8 layouts, partition-major packed
    # e1ts[b, p, sl, hc, i] = emb1[b, sl*512+i, hc*128+p]
    e1ts = np.ascontiguousarray(
        emb1.reshape(B, NSLAB, SLAB, NHC, P).transpose(0, 4, 1, 3, 2)
    ).astype(f8).reshape(B, P, -1)
    # e2ts[b, p, jt, hc, j] = emb2[b, jt*128+j, hc*128+p] * 16*wcq[hc*128+p]
    e2s = emb2 * (WS * w_cq)[None, None, :]
    e2ts = np.ascontiguousarray(
        e2s.reshape(B, NJT, P, NHC, P).transpose(0, 4, 1, 3, 2)
    ).astype(f8).reshape(B, P, -1)
    # e2tt[b, p, g, hc, j] = emb2[b, g*512+j, hc*128+p]  (bf16)
    e2tt = np.ascontiguousarray(
        emb2.reshape(B, NG, SLAB, NHC, P).transpose(0, 4, 1, 3, 2)
    ).astype(bf).reshape(B, P, -1)
    # natural fp8 layouts
    # e1n[b, p, ic, h] = emb1[b, ic*128+p, h]
    e1n = np.ascontiguousarray(
        emb1.reshape(B, NIC, P, H).transpose(0, 2, 1, 3)
    ).astype(f8).reshape(B, P, -1)
    e2n = np.ascontiguousarray(
        emb2.reshape(B, NJT, P, H).transpose(0, 2, 1, 3)
    ).astype(f8).reshape(B, P, -1)

    # weights: w_red rows [w1; w2; w3; w4], scaled x16, OUT->OUTP padded
    wr = w_red.reshape(4, H, OUT) * WS
    w23 = np.zeros((H, 2, OUTP), np.float32)
    w23[:, 0, :OUT] = wr[1]
    w23[:, 1, :OUT] = wr[2]
    w23q = np.ascontiguousarray(
        w23.reshape(NHC, P, 2, OUTP).transpose(1, 0, 2, 3)
    ).astype(f8).reshape(P, -1)
    w14 = np.zeros((H, 2, OUTP), np.float32)
    w14[:, 0, :OUT] = wr[0]
    w14[:, 1, :OUT] = wr[3]
    w14s = np.ascontiguousarray(
        w14.reshape(NHC, P, 2, OUTP).transpose(1, 0, 2, 3)
    ).astype(bf).reshape(P, -1)
    bredp = np.zeros((1, OUTP), np.float32)
    bredp[0, :OUT] = b_red

    in_maps = []
    for b in range(B):
        in_maps.append({
            "e2ts": e2ts[b], "e1ts": e1ts[b], "e1n": e1n[b], "e2n": e2n[b],
            "e2tt": e2tt[b],
            "qrow": np.ascontiguousarray(qrow[b:b + 1]),
            "crow": c_sb[b],
            "w23q": w23q, "w14s": w14s, "bred": bredp,
        })
    return in_maps


def kernel(emb1, emb2, w_c, b_c, w_q, b_q, w_cq, b_cq, w_red, b_red):
    from concourse.bass_utils import run_bass_kernel_spmd

    nc = _get_nc()
    # b_c, b_q, b_cq cancel exactly in both softmaxes (per-row/col consts).
    in_maps = _prep_inputs(emb1, emb2, w_c, w_q, w_cq, w_red, b_red)
    res = run_bass_kernel_spmd(nc, in_maps, core_ids=list(range(NCORES)))
    return np.stack([res.results[i]["out"] for i in range(NCORES)],
                    axis=0).astype(np.float32)
